# revision 17
# baseline (speedup 1.0000x reference)
"""Training-mode BatchNorm2d over x(64,256,56,56) f32 on 8 trn2 NeuronCores.

Sharding: channel-parallel (32 channels per core) — each core owns complete
per-channel reductions, so no cross-core collectives are needed.

The 2e-2 rel-err budget admits a bf16 HBM data path: the host converts x to
bf16 (max rounding error ~2^-9 of value), the device reads bf16, computes
stats in f32, normalizes, and writes bf16 back; the host converts the output
to f32. HBM traffic per core halves to 12.85 MB read + 12.85 MB write
(~63us at the measured per-core aggregate DMA rate) — the floor this kernel
is built around.

Layout: per core 8 channel-blocks of 4 channels; each block is two
half-tiles [128p, 3136] bf16 (partition p = b_lo*4 + cc, half = b_hi), so
16 loads + 16 stores of 800KB. All 16 halves stay resident in SBUF (12.25
MB) between the stats pass and the normalize pass (minimal 2x HBM traffic).

Stats are spread so every engine stays well under the ~7.75us/block DMA
pace (the exact mean and exact variance are both computed — no
approximation beyond the bf16 rounding):
 - per-channel sum(x) for BOTH halves on the (otherwise idle)
   TensorEngine: 7 matmuls per half of x-chunks [128, 448] (moving, bf16)
   against a (1/32)-scaled channel-indicator (stationary, bf16; 1/32 is
   exact), PSUM-accumulated into [4, 448] and folded by one DVE
   reduce_sum.
 - per-partition sum(x^2): half 0 via ScalarE Square activation with
   accum_out (~3.4us); half 1 via DVE tensor_tensor_reduce x*x with
   accum_out, which runs in 2x bf16 mode (~1.7us). Both accumulate f32.
 - the two per-partition sumsq columns are reduced per-channel by one
   tiny f32 matmul.
 - scalar tail (var, rsqrt, A=gamma*rstd, B=beta-mean*A) on DVE (its small
   ops are ~3x cheaper than ACT's); Sqrt on ACT (DVE has none); (A, B)
   broadcast to 128 partitions by a tiny PE matmul.
 - normalize x*A + B in place on DVE for both halves (tensor_scalar, 4x
   bf16 mode, ~1.25us per half).

Per-block engine busy vs the 7.75us DMA window: DVE ~6.1us, ACT ~4.0us,
PE ~6.0us — the DMA stream is the pacer throughout.

The tail of block k is emitted between block k+1's two stat halves, so on
the in-order DVE stream the chain+norms of block k run while block k+1's
half-1 load is still in flight, and the half-1 square starts the moment
the load lands. Input DMAs ride the SP HWDGE ring (no waits ever land
there, so all 16 loads stream back-to-back); output DMAs ride the ACT
HWDGE ring, pushed right after the DVE norms complete.
"""

from contextlib import ExitStack

import ml_dtypes
import numpy as np

import concourse.bass as bass
import concourse.tile as tile
from concourse import bacc, mybir
from concourse.bass_utils import run_bass_kernel_spmd

F32 = mybir.dt.float32
BF16 = mybir.dt.bfloat16
NP_BF16 = np.dtype(ml_dtypes.bfloat16)

B, C, H, W = 64, 256, 56, 56
HW = H * W  # 3136
N_CORES = 8
C_LOC = C // N_CORES  # 32 channels per core
CBLK = 4  # channels per block
N_BLOCKS = C_LOC // CBLK  # 8 blocks per core
BL = 128 // CBLK  # 32 b_lo values packed in the partition dim
BH = B // BL  # 2 half-tiles per block (b_hi)
N_TILE = N_BLOCKS * BH  # 16 tiles per core
SUB = 448  # PE sum-matmul chunk width (3136 = 7*448, <= 512 PSUM cols)
NSUB = HW // SUB  # 7
N_PART = BH * HW  # elems per partition per block = 6272
EPS = 1e-5

_NC_CACHE = {}


def _build_nc(nbufs=16):
    # Bacc (not plain Bass): its finalize() runs generate_event_semaphores,
    # which splits multi-sem waits — TRN2 instructions carry at most one.
    nc = bacc.Bacc()
    x = nc.dram_tensor("x", [N_TILE, 128, HW], BF16, kind="ExternalInput")
    y = nc.dram_tensor("y", [N_TILE, 128, HW], BF16, kind="ExternalOutput")
    gamma = nc.dram_tensor("gamma", [CBLK, N_BLOCKS], F32, kind="ExternalInput")
    beta = nc.dram_tensor("beta", [CBLK, N_BLOCKS], F32, kind="ExternalInput")
    sel8b = nc.dram_tensor("sel8b", [128, CBLK], BF16, kind="ExternalInput")
    sel8f = nc.dram_tensor("sel8f", [128, CBLK], F32, kind="ExternalInput")
    selT = nc.dram_tensor("selT", [CBLK, 128], F32, kind="ExternalInput")

    AF = mybir.ActivationFunctionType
    OP = mybir.AluOpType

    with ExitStack() as ctx:
        tc = ctx.enter_context(tile.TileContext(nc))
        xpool = ctx.enter_context(tc.tile_pool(name="xdata", bufs=nbufs))
        qpool = ctx.enter_context(tc.tile_pool(name="sqscr", bufs=4))
        spool = ctx.enter_context(tc.tile_pool(name="stats", bufs=4))
        cpool = ctx.enter_context(tc.tile_pool(name="const", bufs=1))
        ppool = ctx.enter_context(tc.tile_pool(name="psum", bufs=2, space="PSUM"))
        pspool = ctx.enter_context(tc.tile_pool(name="psums", bufs=3, space="PSUM"))

        sel8b_t = cpool.tile([128, CBLK], BF16)
        nc.gpsimd.dma_start(out=sel8b_t, in_=sel8b[:, :])
        sel8f_t = cpool.tile([128, CBLK], F32)
        nc.gpsimd.dma_start(out=sel8f_t, in_=sel8f[:, :])
        selT_t = cpool.tile([CBLK, 128], F32)
        nc.gpsimd.dma_start(out=selT_t, in_=selT[:, :])
        gam_t = cpool.tile([CBLK, N_BLOCKS], F32)
        nc.gpsimd.dma_start(out=gam_t, in_=gamma[:, :])
        bet_t = cpool.tile([CBLK, N_BLOCKS], F32)
        nc.gpsimd.dma_start(out=bet_t, in_=beta[:, :])
        eps_t = cpool.tile([CBLK, 1], F32)
        nc.vector.memset(eps_t, EPS)

        def sum_mms(psum_s, xt, j):
            xv = xt.rearrange("p (s f) -> p s f", f=SUB)
            for s in range(NSUB):
                nc.tensor.matmul(
                    psum_s,
                    sel8b_t,
                    xv[:, s, :],
                    start=(j == 0 and s == 0),
                    stop=(j == 1 and s == NSUB - 1),
                )

        def stats_block(blk, pqb, pslot):
            """One block: 2 loads + 2 ACT sum(x^2) + PE sum chunks; the
            per-channel sumsq pair lands in pqb columns [2*pslot, 2*pslot+2)."""
            pack = spool.tile([128, 2], F32)
            psum_s = pspool.tile([CBLK, SUB], F32, tag="ps")
            xts = []
            for j in range(BH):
                xt = xpool.tile([128, HW], BF16, tag="x")
                nc.sync.dma_start(out=xt, in_=x[blk * BH + j, :, :])
                xts.append(xt)
                scr = qpool.tile([128, HW], BF16, tag="scr")
                nc.scalar.activation(scr, xt, AF.Square, accum_out=pack[:, j : j + 1])
                sum_mms(psum_s, xt, j)
            nc.tensor.matmul(
                pqb[:, 2 * pslot : 2 * pslot + 2], sel8f_t, pack,
                start=True, stop=True, skip_group_check=True,
            )
            return xts, psum_s

        def norm_pair(pair, blocks):
            """Batched fold + scalar tail + normalize + stores for a PAIR of
            blocks: one chain on [4,2]-wide tiles per two blocks, halving
            the chain executions and amortizing the cross-engine latency."""
            pqb = blocks[0][3]
            # fold PE sums: mean = (sum/32)/6272 per channel, per block col
            s4b = spool.tile([CBLK, 2], F32)
            for j, (blk, xts, psum_s, _) in enumerate(blocks):
                nc.vector.reduce_sum(
                    s4b[:, j : j + 1], psum_s, axis=mybir.AxisListType.X
                )
            meanb = spool.tile([CBLK, 2], F32)
            nc.vector.tensor_scalar_mul(meanb, s4b, 1.0 / N_PART)
            # E[x^2] = (sumsq_h0 + sumsq_h1)/32/6272, per block col
            pq4 = pqb.rearrange("p (b h) -> p b h", h=2)
            e1b = spool.tile([CBLK, 2], F32)
            nc.vector.tensor_scalar_mul(e1b, pq4[:, :, 0], 1.0 / N_PART)
            t2b = spool.tile([CBLK, 2], F32)
            nc.vector.tensor_scalar_mul(t2b, pq4[:, :, 1], 1.0 / N_PART)
            ex2b = spool.tile([CBLK, 2], F32)
            nc.vector.tensor_add(ex2b, e1b, t2b)
            m2b = spool.tile([CBLK, 2], F32)
            nc.vector.tensor_mul(m2b, meanb, meanb)
            varb = spool.tile([CBLK, 2], F32)
            nc.vector.tensor_sub(varb, ex2b, m2b)
            stdb = spool.tile([CBLK, 2], F32)
            nc.scalar.activation(stdb, varb, AF.Sqrt, bias=eps_t)
            rstdb = spool.tile([CBLK, 2], F32)
            nc.vector.reciprocal(rstdb, stdb)
            # A = gamma*rstd, B = beta - mean*A; interleave into [A0,B0,A1,B1]
            b0 = blocks[0][0]
            abB = spool.tile([CBLK, 4], F32)
            ab4 = abB.rearrange("p (b h) -> p b h", h=2)
            nc.vector.tensor_mul(ab4[:, :, 0], rstdb, gam_t[:, b0 : b0 + 2])
            t4b = spool.tile([CBLK, 2], F32)
            nc.vector.tensor_mul(t4b, meanb, ab4[:, :, 0])
            nc.vector.tensor_sub(ab4[:, :, 1], bet_t[:, b0 : b0 + 2], t4b)

            # broadcast (A, B) x 2 blocks to all 128 partitions via PE
            ps2 = ppool.tile([128, 4], F32, tag="pb")
            nc.tensor.matmul(ps2, selT_t, abB, start=True, stop=True)
            ab = spool.tile([128, 4], F32)
            nc.vector.tensor_copy(ab, ps2)

            # normalize all 4 halves on DVE; stores ride the ACT HWDGE ring
            for j, (blk, xts, psum_s, _) in enumerate(blocks):
                for q, xt in enumerate(xts):
                    nc.vector.tensor_scalar(
                        out=xt, in0=xt,
                        scalar1=ab[:, 2 * j : 2 * j + 1],
                        scalar2=ab[:, 2 * j + 1 : 2 * j + 2],
                        op0=OP.mult, op1=OP.add,
                    )
                    nc.scalar.dma_start(out=y[blk * BH + q, :, :], in_=xt)

        # Software pipeline over block pairs: pair i's tail is emitted
        # between pair i+1's two stats blocks, so the sqrt lands mid-queue
        # on ACT (already satisfiable) and the DVE chain+norms run while
        # pair i+1's squares stream on ACT.
        N_PAIR = N_BLOCKS // 2
        prev = None
        for pair in range(N_PAIR):
            b0 = 2 * pair
            pqb = ppool.tile([CBLK, 4], F32, tag="pq")
            blk_a = stats_block(b0, pqb, 0)
            if prev is not None:
                norm_pair(*prev)
                prev = None
            blk_b = stats_block(b0 + 1, pqb, 1)
            blocks = [
                (b0, blk_a[0], blk_a[1], pqb),
                (b0 + 1, blk_b[0], blk_b[1], pqb),
            ]
            if pair == 0:
                norm_pair(pair, blocks)
            else:
                prev = (pair, blocks)
        if prev is not None:
            norm_pair(*prev)
    nc.finalize()
    return nc


def get_nc(nbufs=16):
    if nbufs not in _NC_CACHE:
        _NC_CACHE[nbufs] = _build_nc(nbufs)
    return _NC_CACHE[nbufs]


def _sel_matrices():
    # the 1/32 channel-indicator: reduce-matmuls on per-partition values
    # yield (sum over the channel's 32 partitions)/32
    sel = np.zeros((128, CBLK), dtype=np.float32)
    sel[np.arange(128), np.arange(128) % CBLK] = 1.0 / BL
    selT = np.zeros((CBLK, 128), dtype=np.float32)
    selT[np.arange(128) % CBLK, np.arange(128)] = 1.0
    return sel, selT


def pack_inputs(x, gamma, beta):
    """Full f32 inputs -> list of per-core in_maps (bf16 device layout)."""
    x16 = np.asarray(x, dtype=np.float32).astype(NP_BF16)
    gamma = np.asarray(gamma, dtype=np.float32)
    beta = np.asarray(beta, dtype=np.float32)
    # [b_hi, b_lo, core, blk, cc, hw] -> [core, blk, b_hi, b_lo, cc, hw]
    xr = np.ascontiguousarray(
        x16.reshape(BH, BL, N_CORES, N_BLOCKS, CBLK, HW).transpose(2, 3, 0, 1, 4, 5)
    )
    g = gamma.reshape(N_CORES, N_BLOCKS, CBLK)
    bt = beta.reshape(N_CORES, N_BLOCKS, CBLK)
    sel, selT = _sel_matrices()
    sel8b = sel.astype(NP_BF16)  # 1/32 is exact in bf16
    in_maps = []
    for i in range(N_CORES):
        in_maps.append(
            {
                "x": xr[i].reshape(N_TILE, 128, HW),
                "gamma": np.ascontiguousarray(g[i].T),
                "beta": np.ascontiguousarray(bt[i].T),
                "sel8b": sel8b,
                "sel8f": sel,
                "selT": selT,
            }
        )
    return in_maps


def unpack_outputs(per_core_y):
    """List of per-core y (bf16 device layout) -> full f32 (64,256,56,56)."""
    ys = np.stack(per_core_y)  # [core, blk*b_hi, 128, hw] bf16
    out = (
        ys.reshape(N_CORES, N_BLOCKS, BH, BL, CBLK, HW)
        .transpose(2, 3, 0, 1, 4, 5)
        .astype(np.float32)
        .reshape(B, C, H, W)
    )
    return out


def run(inputs, trace=False, nbufs=16):
    """Returns (full_output, BassKernelResults)."""
    nc = get_nc(nbufs)
    in_maps = pack_inputs(inputs["x"], inputs["gamma"], inputs["beta"])
    res = run_bass_kernel_spmd(nc, in_maps, list(range(N_CORES)), trace=trace)
    out = unpack_outputs([r["y"] for r in res.results])
    return out, res


def kernel(**inputs):
    out, _ = run(inputs)
    return out


# revision 20
# speedup vs baseline: 1.1220x; 1.1220x over previous
"""Training-mode BatchNorm2d over x(64,256,56,56) f32 on 8 trn2 NeuronCores.

Sharding: channel-parallel (32 channels per core) — each core owns complete
per-channel reductions, so no cross-core collectives are needed.

The 2e-2 rel-err budget admits a bf16 HBM data path: the host converts x to
bf16 (max rounding error ~2^-9 of value), the device reads bf16, computes
stats in f32, normalizes, and writes bf16 back; the host converts the output
to f32. HBM traffic per core halves to 12.85 MB read + 12.85 MB write
(~63us at the measured per-core aggregate DMA rate) — the floor this kernel
is built around.

Layout: per core 8 channel-blocks of 4 channels; each block is two
half-tiles [128p, 3136] bf16 (partition p = b_lo*4 + cc, half = b_hi), so
16 loads + 16 stores of 800KB. All 16 halves stay resident in SBUF (12.25
MB) between the stats pass and the normalize pass (minimal 2x HBM traffic).

Stats are spread across ALL FIVE engines so none exceeds the ~7.75us/block
DMA pace (exact mean and exact variance — no approximation beyond bf16
rounding):
 - per-channel sum(x) for BOTH halves on the TensorEngine: 7 matmuls per
   half of x-chunks [128, 448] (moving, bf16) against a (1/32)-scaled
   channel-indicator (stationary, bf16; 1/32 is exact), PSUM-accumulated
   into [4, 448] and folded by one DVE reduce_sum.
 - per-partition sum(x^2): half 0 via ScalarE Square activation with
   accum_out (~3.2us); half 1 via GpSimd scalar_tensor_tensor x*x with
   accum_out (software Q7 op — the engine is otherwise idle). Both
   accumulate f32.
 - the two per-partition sumsq columns are reduced per-channel by one
   tiny f32 matmul.
 - scalar tail (var, rsqrt, A=gamma*rstd, B=beta-mean*A) on DVE (its small
   ops are ~3x cheaper than ACT's); Sqrt on ACT (DVE has none); (A, B)
   broadcast to 128 partitions by a tiny PE matmul.
 - normalize x*A + B in place on DVE for both halves (tensor_scalar, 4x
   bf16 mode, ~0.9us per half).

The tail of block k is emitted between block k+1's two stat halves, so on
the in-order streams the chain of block k runs while block k+1's half-0
square streams on ACT and its half-1 load is in flight. Input DMAs ride
the SP HWDGE ring (no waits ever land there, so all 16 loads stream
back-to-back); output DMAs ride the ACT HWDGE ring, pushed right after
the DVE norms complete.
"""

from contextlib import ExitStack

import ml_dtypes
import numpy as np

import concourse.bass as bass
import concourse.tile as tile
from concourse import bacc, mybir
from concourse.bass_utils import run_bass_kernel_spmd

F32 = mybir.dt.float32
BF16 = mybir.dt.bfloat16
NP_BF16 = np.dtype(ml_dtypes.bfloat16)

B, C, H, W = 64, 256, 56, 56
HW = H * W  # 3136
N_CORES = 8
C_LOC = C // N_CORES  # 32 channels per core
CBLK = 4  # channels per block
N_BLOCKS = C_LOC // CBLK  # 8 blocks per core
BL = 128 // CBLK  # 32 b_lo values packed in the partition dim
BH = B // BL  # 2 half-tiles per block (b_hi)
N_TILE = N_BLOCKS * BH  # 16 tiles per core
SUB = 448  # PE sum-matmul chunk width (3136 = 7*448, <= 512 PSUM cols)
NSUB = HW // SUB  # 7
N_PART = BH * HW  # elems per partition per block = 6272
EPS = 1e-5

_NC_CACHE = {}


def _build_nc(nbufs=16):
    # Bacc (not plain Bass): its finalize() runs generate_event_semaphores,
    # which splits multi-sem waits — TRN2 instructions carry at most one.
    nc = bacc.Bacc()
    x = nc.dram_tensor("x", [N_TILE, 128, HW], BF16, kind="ExternalInput")
    y = nc.dram_tensor("y", [N_TILE, 128, HW], BF16, kind="ExternalOutput")
    gamma = nc.dram_tensor("gamma", [CBLK, N_BLOCKS], F32, kind="ExternalInput")
    beta = nc.dram_tensor("beta", [CBLK, N_BLOCKS], F32, kind="ExternalInput")
    sel8b = nc.dram_tensor("sel8b", [128, CBLK], BF16, kind="ExternalInput")
    sel8f = nc.dram_tensor("sel8f", [128, CBLK], F32, kind="ExternalInput")
    selT = nc.dram_tensor("selT", [CBLK, 128], F32, kind="ExternalInput")

    AF = mybir.ActivationFunctionType
    OP = mybir.AluOpType

    with ExitStack() as ctx:
        tc = ctx.enter_context(tile.TileContext(nc))
        xpool = ctx.enter_context(tc.tile_pool(name="xdata", bufs=nbufs))
        qpool = ctx.enter_context(tc.tile_pool(name="sqscr", bufs=4))
        spool = ctx.enter_context(tc.tile_pool(name="stats", bufs=4))
        cpool = ctx.enter_context(tc.tile_pool(name="const", bufs=1))
        ppool = ctx.enter_context(tc.tile_pool(name="psum", bufs=2, space="PSUM"))

        sel8b_t = cpool.tile([128, CBLK], BF16)
        nc.gpsimd.dma_start(out=sel8b_t, in_=sel8b[:, :])
        sel8f_t = cpool.tile([128, CBLK], F32)
        nc.gpsimd.dma_start(out=sel8f_t, in_=sel8f[:, :])
        selT_t = cpool.tile([CBLK, 128], F32)
        nc.gpsimd.dma_start(out=selT_t, in_=selT[:, :])
        gam_t = cpool.tile([CBLK, N_BLOCKS], F32)
        nc.gpsimd.dma_start(out=gam_t, in_=gamma[:, :])
        bet_t = cpool.tile([CBLK, N_BLOCKS], F32)
        nc.gpsimd.dma_start(out=bet_t, in_=beta[:, :])
        eps_t = cpool.tile([CBLK, 1], F32)
        nc.vector.memset(eps_t, EPS)

        def sum_mms(psum_s, xt, j):
            xv = xt.rearrange("p (s f) -> p s f", f=SUB)
            for s in range(NSUB):
                nc.tensor.matmul(
                    psum_s,
                    sel8b_t,
                    xv[:, s, :],
                    start=(j == 0 and s == 0),
                    stop=(j == 1 and s == NSUB - 1),
                )

        def stats_phase_a(blk):
            """Half 0: load + ACT sum(x^2) + PE sum chunks."""
            pack = spool.tile([128, 2], F32)
            psum_s = ppool.tile([CBLK, SUB], F32, tag="ps")
            xt0 = xpool.tile([128, HW], BF16, tag="x")
            nc.sync.dma_start(out=xt0, in_=x[blk * BH, :, :])
            scr = qpool.tile([128, HW], BF16, tag="scra")
            nc.scalar.activation(scr, xt0, AF.Square, accum_out=pack[:, 0:1])
            sum_mms(psum_s, xt0, 0)
            return xt0, pack, psum_s

        def stats_phase_b(blk, xt0, pack, psum_s):
            """Half 1: load + GpSimd sum(x^2) + PE sum chunks + reduce."""
            xt1 = xpool.tile([128, HW], BF16, tag="x")
            nc.sync.dma_start(out=xt1, in_=x[blk * BH + 1, :, :])
            scr = qpool.tile([128, HW], BF16, tag="scrv")
            nc.vector.scalar_tensor_tensor(
                out=scr,
                in0=xt1,
                scalar=1.0,
                in1=xt1,
                op0=OP.mult,
                op1=OP.mult,
                accum_out=pack[:, 1:2],
            )
            sum_mms(psum_s, xt1, 1)
            # PE: per-channel [sumsq_h0, sumsq_h1] / 32
            pq = ppool.tile([CBLK, 2], F32, tag="pq")
            nc.tensor.matmul(pq, sel8f_t, pack, start=True, stop=True)
            return xt0, xt1, psum_s, pq

        def norm_phase(blk, xt0, xt1, psum_s, pq):
            """Fold + scalar tail (almost all on ACT — small-op latency
            hides under its slack; only reciprocal is DVE) + normalize on
            DVE. Emitted between the next block's two stat halves so the
            cross-engine round-trips hide under its streaming work.
            Returns the store closures, pushed one block later so the ACT
            stream never waits on a norm semaphore."""
            # fold PE sums on ACT: s4 = (sum/32) per channel, via
            # Copy-with-accumulate from PSUM; scr4 is a throwaway
            scr4 = spool.tile([CBLK, SUB], F32)
            s4 = spool.tile([CBLK, 1], F32)
            nc.scalar.activation(scr4, psum_s, AF.Copy, accum_out=s4)
            mean = spool.tile([CBLK, 1], F32)
            nc.scalar.activation(mean, s4, AF.Copy, scale=1.0 / N_PART)
            # E[x^2] = (sumsq_h0 + sumsq_h1)/32/6272
            e1 = spool.tile([CBLK, 1], F32)
            nc.scalar.activation(e1, pq[:, 0:1], AF.Copy, scale=1.0 / N_PART)
            ex2 = spool.tile([CBLK, 1], F32)
            nc.scalar.activation(
                ex2, pq[:, 1:2], AF.Identity, scale=1.0 / N_PART, bias=e1
            )
            m2b = spool.tile([CBLK, 1], F32)
            nc.scalar.activation(m2b, mean, AF.Square)
            var = spool.tile([CBLK, 1], F32)
            nc.scalar.activation(var, m2b, AF.Identity, scale=-1.0, bias=ex2)
            std = spool.tile([CBLK, 1], F32)
            nc.scalar.activation(std, var, AF.Sqrt, bias=eps_t)
            rstd = spool.tile([CBLK, 1], F32)
            nc.vector.reciprocal(rstd, std)
            # A = gamma*rstd, B = beta - mean*A
            ab8 = spool.tile([CBLK, 2], F32)
            nc.scalar.activation(
                ab8[:, 0:1], rstd, AF.Copy, scale=gam_t[:, blk : blk + 1]
            )
            t4 = spool.tile([CBLK, 1], F32)
            nc.scalar.activation(t4, mean, AF.Copy, scale=ab8[:, 0:1])
            nc.scalar.activation(
                ab8[:, 1:2], t4, AF.Identity, scale=-1.0,
                bias=bet_t[:, blk : blk + 1],
            )

            # broadcast (A, B) to all 128 partitions via PE matmul
            ps2 = ppool.tile([128, 2], F32, tag="pb")
            nc.tensor.matmul(ps2, selT_t, ab8, start=True, stop=True)
            ab = spool.tile([128, 2], F32)
            nc.scalar.activation(ab, ps2, AF.Copy)

            # normalize both halves on DVE
            for xt in (xt0, xt1):
                nc.vector.tensor_scalar(
                    out=xt, in0=xt, scalar1=ab[:, 0:1], scalar2=ab[:, 1:2],
                    op0=OP.mult, op1=OP.add,
                )

            def push_stores():
                nc.scalar.dma_start(out=y[blk * BH, :, :], in_=xt0)
                nc.scalar.dma_start(out=y[blk * BH + 1, :, :], in_=xt1)

            return push_stores

        # Software pipeline: the tail of block k is emitted between block
        # k+1's two stat halves; its stores are pushed another block later
        # (their norm semaphores are long done by then, so the ACT stream
        # never stalls on them).
        prev = None
        pending_stores = None
        for blk in range(N_BLOCKS):
            a = stats_phase_a(blk)
            if pending_stores is not None:
                pending_stores()
                pending_stores = None
            if blk == 0:
                cur = stats_phase_b(blk, *a)
                pending_stores = norm_phase(blk, *cur)
                prev = None
            else:
                if prev is not None:
                    pending_stores = norm_phase(prev[0], *prev[1])
                cur = stats_phase_b(blk, *a)
                prev = (blk, cur)
        if prev is not None:
            pending = norm_phase(prev[0], *prev[1])
            pending_stores()
            pending()
        elif pending_stores is not None:
            pending_stores()
    nc.finalize()
    return nc


def get_nc(nbufs=16):
    if nbufs not in _NC_CACHE:
        _NC_CACHE[nbufs] = _build_nc(nbufs)
    return _NC_CACHE[nbufs]


def _sel_matrices():
    # the 1/32 channel-indicator: reduce-matmuls on per-partition values
    # yield (sum over the channel's 32 partitions)/32
    sel = np.zeros((128, CBLK), dtype=np.float32)
    sel[np.arange(128), np.arange(128) % CBLK] = 1.0 / BL
    selT = np.zeros((CBLK, 128), dtype=np.float32)
    selT[np.arange(128) % CBLK, np.arange(128)] = 1.0
    return sel, selT


def pack_inputs(x, gamma, beta):
    """Full f32 inputs -> list of per-core in_maps (bf16 device layout)."""
    x16 = np.asarray(x, dtype=np.float32).astype(NP_BF16)
    gamma = np.asarray(gamma, dtype=np.float32)
    beta = np.asarray(beta, dtype=np.float32)
    # [b_hi, b_lo, core, blk, cc, hw] -> [core, blk, b_hi, b_lo, cc, hw]
    xr = np.ascontiguousarray(
        x16.reshape(BH, BL, N_CORES, N_BLOCKS, CBLK, HW).transpose(2, 3, 0, 1, 4, 5)
    )
    g = gamma.reshape(N_CORES, N_BLOCKS, CBLK)
    bt = beta.reshape(N_CORES, N_BLOCKS, CBLK)
    sel, selT = _sel_matrices()
    sel8b = sel.astype(NP_BF16)  # 1/32 is exact in bf16
    in_maps = []
    for i in range(N_CORES):
        in_maps.append(
            {
                "x": xr[i].reshape(N_TILE, 128, HW),
                "gamma": np.ascontiguousarray(g[i].T),
                "beta": np.ascontiguousarray(bt[i].T),
                "sel8b": sel8b,
                "sel8f": sel,
                "selT": selT,
            }
        )
    return in_maps


def unpack_outputs(per_core_y):
    """List of per-core y (bf16 device layout) -> full f32 (64,256,56,56)."""
    ys = np.stack(per_core_y)  # [core, blk*b_hi, 128, hw] bf16
    out = (
        ys.reshape(N_CORES, N_BLOCKS, BH, BL, CBLK, HW)
        .transpose(2, 3, 0, 1, 4, 5)
        .astype(np.float32)
        .reshape(B, C, H, W)
    )
    return out


def run(inputs, trace=False, nbufs=16):
    """Returns (full_output, BassKernelResults)."""
    nc = get_nc(nbufs)
    in_maps = pack_inputs(inputs["x"], inputs["gamma"], inputs["beta"])
    res = run_bass_kernel_spmd(nc, in_maps, list(range(N_CORES)), trace=trace)
    out = unpack_outputs([r["y"] for r in res.results])
    return out, res


def kernel(**inputs):
    out, _ = run(inputs)
    return out
